# revision 33
# baseline (speedup 1.0000x reference)
"""Trainium2 Bass kernel for nn_LstmClassifier: batch-sharded LSTM over 8 cores.

Batch-major fp8-DoubleRow design:
    gates[b, g] = h.T(stationary, hidden-major fp8 x16) @ WcT(moving fp8 x256)
    - h hidden-major == the DR stationary layout, so only h_new (batch-major,
      produced by the pointwise chain) needs an 8-tile PE transpose per step.
    - gate bias enters PSUM via K=128 ones-row matmuls (free-dim bias is not
      expressible on ACT/DVE affordably); psum carries raw*4096, ACT applies
      scale=1/4096.
    - steps 0..WARM-1 run the gates in bf16 (fp8 error is only visible at the
      start where |out| is max); outproj flips to fp8 at s=WARM-1.
    - pointwise is bf16 (DVE 2x) batch-major: sigmoid merged over [i,f] pairs.
"""
import sys
import types
import numpy as np

sys.path.insert(0, "/opt/trn_rl_repo")

import ml_dtypes  # noqa: E402
import concourse.bass as bass  # noqa: E402
import concourse.tile as tile  # noqa: E402
from concourse import bacc, mybir  # noqa: E402
from concourse.bass_utils import run_bass_kernel_spmd  # noqa: E402

B, IN_DIM, HID, OUT_DIM, T = 2048, 1024, 512, 256, 64
NCORES = 8
BSH = B // NCORES          # 256 batch rows per core
KI = IN_DIM // 128         # 8 input k-tiles
WARM = 4                   # steps with bf16 gates (fp8 from step WARM on)
TAIL_FP8 = None            # None = auto (steps-1 >= WARM-1); override for probes
NO_BIAS_MM = False         # probe: drop bias matmuls in fp8 path
F32 = mybir.dt.float32
BF16 = mybir.dt.bfloat16
F16 = mybir.dt.float16
F8 = mybir.dt.float8e4
AF = mybir.ActivationFunctionType
DRM = mybir.MatmulPerfMode.DoubleRow
SW, SH = 256.0, 16.0       # fp8 scales: weights x256, hidden x16
SCALE = 1.0 / (SW * SH)    # psum carries raw*4096 in both fp8 and warm paths

LAST_EXEC_NS = None


def _install_ntff_hook():
    try:
        import antenv.axon_hooks  # noqa: F401
        return True
    except ImportError:
        pass
    try:
        if "/root/.axon_site" not in sys.path:
            sys.path.insert(0, "/root/.axon_site")
        from trn_agent_boot.trn_boot import _ntff_profile_via_ctypes
        hook = _ntff_profile_via_ctypes("/opt/axon/libaxon_pjrt.so")
        if hook is None:
            return False
        import antenv
        mod = types.ModuleType("antenv.axon_hooks")
        mod._hook = hook
        mod.get_axon_ntff_profile_hook = lambda: mod._hook
        mod.set_axon_ntff_profile_hook = lambda h: setattr(mod, "_hook", h)
        antenv.axon_hooks = mod
        sys.modules["antenv.axon_hooks"] = mod
        return True
    except Exception:
        return False


def build_program(steps=T):
    nc = bacc.Bacc("TRN2", target_bir_lowering=False, debug=False)

    xT_d = nc.dram_tensor("xT", [128, KI, BSH], BF16, kind="ExternalInput").ap()
    winT_d = nc.dram_tensor("winT", [128, KI, HID], BF16, kind="ExternalInput").ap()
    binp_d = nc.dram_tensor("binp", [128, 4], F32, kind="ExternalInput").ap()
    wc16_d = nc.dram_tensor("wc16", [128, 4, 4 * HID], BF16, kind="ExternalInput").ap()
    wc8_d = nc.dram_tensor("wc8", [128, 2, 4 * HID, 2], F8, kind="ExternalInput").ap()
    wo16_d = nc.dram_tensor("wo16", [128, 4, OUT_DIM], BF16, kind="ExternalInput").ap()
    wo8_d = nc.dram_tensor("wo8", [128, 2, OUT_DIM, 2], F8, kind="ExternalInput").ap()
    biasb_d = nc.dram_tensor("biasb", [128, 4 * HID], BF16, kind="ExternalInput").ap()
    boutb_d = nc.dram_tensor("boutb", [128, 2, OUT_DIM], BF16, kind="ExternalInput").ap()
    onesw_d = nc.dram_tensor("onesw", [128, 128], BF16, kind="ExternalInput").ap()
    ones8_d = nc.dram_tensor("ones8", [128, 2, 128], F8, kind="ExternalInput").ap()
    biasb8_d = nc.dram_tensor("biasb8", [128, 4 * HID, 2], F8, kind="ExternalInput").ap()
    boutb8_d = nc.dram_tensor("boutb8", [128, 2 * OUT_DIM, 2], F8, kind="ExternalInput").ap()
    ident_d = nc.dram_tensor("ident", [128, 128], BF16, kind="ExternalInput").ap()
    out_d = nc.dram_tensor("out", [BSH, steps, OUT_DIM], F16, kind="ExternalOutput").ap()
    # [p, bh, t, o]: batch row = bh*128 + p
    out_v = out_d.rearrange("(m p) t o -> p m t o", p=128)

    from contextlib import ExitStack
    with tile.TileContext(nc) as tc, ExitStack() as ctx:
        consts = ctx.enter_context(tc.tile_pool(name="consts", bufs=1))
        h16p = ctx.enter_context(tc.tile_pool(name="h16", bufs=2))
        h8p = ctx.enter_context(tc.tile_pool(name="h8", bufs=2))
        cpool = ctx.enter_context(tc.tile_pool(name="c", bufs=2))
        hbp = ctx.enter_context(tc.tile_pool(name="hb", bufs=2))
        spool = ctx.enter_context(tc.tile_pool(name="s", bufs=4))
        dpool = ctx.enter_context(tc.tile_pool(name="d", bufs=6))
        opool = ctx.enter_context(tc.tile_pool(name="osb", bufs=3))
        gp = ctx.enter_context(tc.tile_pool(name="gp", bufs=3, space="PSUM"))
        tpp = ctx.enter_context(tc.tile_pool(name="tp", bufs=1, space="PSUM"))
        opp = ctx.enter_context(tc.tile_pool(name="op", bufs=1, space="PSUM"))

        # ---- load constants ----
        xT = consts.tile([128, KI, BSH], BF16)
        nc.sync.dma_start(xT[:], xT_d[:])
        winT = consts.tile([128, KI, HID], BF16)
        nc.sync.dma_start(winT[:], winT_d[:])
        binp = consts.tile([128, 4], F32)
        nc.sync.dma_start(binp[:], binp_d[:])
        wc16 = consts.tile([128, 4, 4 * HID], BF16)
        nc.sync.dma_start(wc16[:], wc16_d[:])
        wc8 = consts.tile([128, 2, 4 * HID, 2], F8)
        nc.sync.dma_start(wc8[:], wc8_d[:])
        wo16 = consts.tile([128, 4, OUT_DIM], BF16)
        nc.sync.dma_start(wo16[:], wo16_d[:])
        wo8 = consts.tile([128, 2, OUT_DIM, 2], F8)
        nc.sync.dma_start(wo8[:], wo8_d[:])
        biasb = consts.tile([128, 4 * HID], BF16)
        nc.sync.dma_start(biasb[:], biasb_d[:])
        boutb = consts.tile([128, 2, OUT_DIM], BF16)
        nc.sync.dma_start(boutb[:], boutb_d[:])
        onesw = consts.tile([128, 128], BF16)
        nc.sync.dma_start(onesw[:], onesw_d[:])
        ones8 = consts.tile([128, 2, 128], F8)
        nc.sync.dma_start(ones8[:], ones8_d[:])
        biasb8 = consts.tile([128, 4 * HID, 2], F8)
        nc.sync.dma_start(biasb8[:], biasb8_d[:])
        boutb8 = consts.tile([128, 2 * OUT_DIM, 2], F8)
        nc.sync.dma_start(boutb8[:], boutb8_d[:])
        ident = consts.tile([128, 128], BF16)
        nc.sync.dma_start(ident[:], ident_d[:])

        # ---- input projection: h0.T = relu(W_in @ x.T + b_in), hidden-major ----
        hs16 = h16p.tile([128, 4, BSH], BF16, tag="h16", name="hs16_init")
        ph = gp.tile([128, 2, HID], F32, tag="g", name="ph0")
        for m in range(4):
            for k in range(KI):
                nc.tensor.matmul(
                    ph[:, m // 2, (m % 2) * 256:(m % 2) * 256 + 256],
                    winT[:, k, m * 128:(m + 1) * 128],
                    xT[:, k, :],
                    start=(k == 0 and m % 2 == 0),
                    stop=(k == KI - 1 and m % 2 == 1),
                )
        for m in range(4):
            nc.scalar.activation(
                hs16[:, m, :], ph[:, m // 2, (m % 2) * 256:(m % 2) * 256 + 256],
                AF.Relu, bias=binp[:, m:m + 1],
            )

        hs8 = [None, None]   # per-bh stationary fp8 tiles
        c = None
        po = None

        def emit_outproj_drain(t_idx, po_t):
            osb = opool.tile([128, 2, OUT_DIM], F16, tag="osb", name=f"osb{t_idx}")
            nc.vector.tensor_scalar_mul(osb[:], po_t[:], SCALE)
            nc.sync.dma_start(out_v[:, :, t_idx, :], osb[:])

        def emit_phase(t, bh, fp8, po, cur8, cur16):
            """matmul phase for one batch half: bias + gates + outproj(t-1)."""
            psA = gp.tile([128, 2, 512], F32, tag="g", name=f"gA{t}_{bh}")
            psB = gp.tile([128, 2, 512], F32, tag="g", name=f"gB{t}_{bh}")
            chunks = [(psA, 0, 0), (psA, 1, 1), (psB, 0, 2), (psB, 1, 3)]
            # bias rows start each bank's accumulation group (mode must
            # match the gate matmuls: mixing normal+DR in a group hangs PE)
            for ps, slot, ch in chunks:
                if fp8:
                    nc.tensor.matmul(
                        ps[:, slot, :], ones8[:],
                        biasb8[:, ch * 512:(ch + 1) * 512, :]
                        .rearrange("p n two -> p two n"),
                        start=True, stop=False, perf_mode=DRM,
                    )
                else:
                    nc.tensor.matmul(
                        ps[:, slot, :], onesw[:], biasb[:, ch * 512:(ch + 1) * 512],
                        start=True, stop=False,
                    )
            if fp8:
                for kc in range(2):
                    lhs = cur8[bh][:, kc, :, :]
                    for ps, slot, ch in chunks:
                        nc.tensor.matmul(
                            ps[:, slot, :], lhs,
                            wc8[:, kc, ch * 512:(ch + 1) * 512, :]
                            .rearrange("p n two -> p two n"),
                            start=False, stop=(kc == 1),
                            perf_mode=DRM,
                        )
                    if po is not None:
                        nc.tensor.matmul(
                            po[:, bh, :], lhs,
                            wo8[:, kc, :, :].rearrange("p n two -> p two n"),
                            start=False,
                            stop=(kc == 1 and bh == 1),
                            perf_mode=DRM,
                        )
            else:
                for k in range(4):
                    lhs = cur16[:, k, bh * 128:(bh + 1) * 128]
                    for ps, slot, ch in chunks:
                        nc.tensor.matmul(
                            ps[:, slot, :], lhs,
                            wc16[:, k, ch * 512:(ch + 1) * 512],
                            start=False, stop=(k == 3),
                        )
                    if po is not None:
                        nc.tensor.matmul(
                            po[:, bh, :], lhs, wo16[:, k, :],
                            start=False, stop=(k == 3 and bh == 1),
                        )
            return psA, psB

        def emit_sigma(t, bh, psA, psB):
            sA = spool.tile([128, 2, 512], BF16, tag="sA", name=f"sA{t}_{bh}")
            sB = spool.tile([128, 2, 512], BF16, tag="sB", name=f"sB{t}_{bh}")
            nc.scalar.activation(sA[:], psA[:], AF.Sigmoid, scale=SCALE)
            nc.scalar.activation(sB[:, 0, :], psB[:, 0, :], AF.Sigmoid, scale=SCALE)
            nc.scalar.activation(sB[:, 1, :], psB[:, 1, :], AF.Tanh, scale=SCALE)
            return sA, sB

        def emit_cell(t, bh, sA, sB, cn):
            if t == 0:
                nc.vector.tensor_mul(cn[:, bh, :], sA[:, 0, :], sB[:, 1, :])
            else:
                tmp = dpool.tile([128, 512], BF16, tag="tmp", name=f"tmp{t}_{bh}")
                nc.vector.tensor_mul(tmp[:], sA[:, 0, :], sB[:, 1, :])
                nc.vector.tensor_mul(cn[:, bh, :], sA[:, 1, :], c[:, bh, :])
                nc.vector.tensor_add(cn[:, bh, :], cn[:, bh, :], tmp[:])

        def emit_h(t, bh, sB, cn, hb, hs8n, hs16n):
            tcb = dpool.tile([128, 512], BF16, tag="tc", name=f"tc{t}_{bh}")
            nc.scalar.activation(tcb[:], cn[:, bh, :], AF.Tanh)
            # hb carries 16*h (the fp8 hidden scale), folded into this mul
            nc.vector.scalar_tensor_tensor(
                hb[:, bh, :], sB[:, 0, :], SH, tcb[:],
                mybir.AluOpType.mult, mybir.AluOpType.mult,
            )
            # transpose h_new -> hidden-major: own-bank tile per bh so the
            # start=True bank-clear can't race the other half's readers
            tp = tpp.tile([128, 2, 4, 128], BF16, tag="tp", name=f"tp{t}_{bh}")
            for ht in range(4):
                nc.tensor.matmul(
                    tp[:, 0, ht, :], hb[:, bh, ht * 128:(ht + 1) * 128],
                    ident[:],
                    start=(ht == 0), stop=(ht == 3),
                    is_transpose=True,
                )
            # cast/copy into next stationary (hb already carries 16*h)
            if hs8n is not None:
                nc.vector.tensor_copy(hs8n[:], tp[:, 0, :, :])
            else:
                nc.vector.tensor_copy(
                    hs16n[:, :, bh * 128:(bh + 1) * 128], tp[:, 0, :, :])

        for t in range(steps):
            fp8 = t >= WARM
            # outproj(t-1) accumulator: bias first (starts the bank)
            if t >= 1:
                po = opp.tile([128, 2, OUT_DIM], F32, tag="po", name=f"po{t-1}")
                if fp8:
                    nc.tensor.matmul(po[:], ones8[:],
                                     boutb8[:].rearrange("p n two -> p two n"),
                                     start=True, stop=False, perf_mode=DRM)
                else:
                    nc.tensor.matmul(po[:], onesw[:], boutb[:], start=True, stop=False)
            else:
                po = None

            cn = cpool.tile([128, 2, HID], BF16, tag="c", name=f"c{t}")
            hb = hbp.tile([128, 2, HID], BF16, tag="hb", name=f"hb{t}")
            if t >= WARM - 1:
                hs8n = [h8p.tile([128, 2, 2, 128], F8, tag=f"h8{b}",
                                 name=f"hs8_{t}_{b}") for b in range(2)]
                hs16n = None
            else:
                hs8n = [None, None]
                hs16n = h16p.tile([128, 4, BSH], BF16, tag="h16", name=f"hs16_{t}")

            # matmul phases (PE stream): b0 then b1
            psA0, psB0 = emit_phase(t, 0, fp8, po, hs8, hs16)
            sA0, sB0 = emit_sigma(t, 0, psA0, psB0)
            psA1, psB1 = emit_phase(t, 1, fp8, po, hs8, hs16)
            sA1, sB1 = emit_sigma(t, 1, psA1, psB1)
            # pointwise + transpose b0 (overlaps b1 matmuls on PE)
            emit_cell(t, 0, sA0, sB0, cn)
            emit_cell(t, 1, sA1, sB1, cn)
            emit_h(t, 0, sB0, cn, hb, hs8n[0], hs16n)
            emit_h(t, 1, sB1, cn, hb, hs8n[1], hs16n)

            if t >= 1:
                emit_outproj_drain(t - 1, po)
            c = cn
            if t >= WARM - 1:
                hs8 = hs8n
            else:
                hs16 = hs16n

        # ---- tail: outproj(T-1) from the last stationary ----
        po = opp.tile([128, 2, OUT_DIM], F32, tag="po", name=f"po{steps-1}")
        tail_fp8 = (steps - 1 >= WARM - 1) if TAIL_FP8 is None else TAIL_FP8
        if tail_fp8:
            nc.tensor.matmul(po[:], ones8[:],
                             boutb8[:].rearrange("p n two -> p two n"),
                             start=True, stop=False, perf_mode=DRM)
        else:
            nc.tensor.matmul(po[:], onesw[:], boutb[:], start=True, stop=False)
        for bh in range(2):
            if tail_fp8:
                for kc in range(2):
                    lhs = hs8[bh][:, kc, :, :]
                    nc.tensor.matmul(
                        po[:, bh, :], lhs,
                        wo8[:, kc, :, :].rearrange("p n two -> p two n"),
                        start=False, stop=(kc == 1 and bh == 1), perf_mode=DRM,
                    )
            else:
                for k in range(4):
                    lhs = hs16[:, k, bh * 128:(bh + 1) * 128]
                    nc.tensor.matmul(
                        po[:, bh, :], lhs, wo16[:, k, :],
                        start=False, stop=(k == 3 and bh == 1),
                    )
        emit_outproj_drain(steps - 1, po)

    nc.compile()
    return nc


_PROGRAM = None


def _get_program():
    global _PROGRAM
    if _PROGRAM is None:
        _PROGRAM = build_program()
    return _PROGRAM


def _pack_inputs(x, W_in, b_in, W_ih, b_ih, W_hh, b_hh, W_out, b_out):
    f32 = np.float32
    bf16 = ml_dtypes.bfloat16
    f8 = ml_dtypes.float8_e4m3fn
    Wc = np.asarray(W_ih, f32) + np.asarray(W_hh, f32)
    bc = np.asarray(b_ih, f32) + np.asarray(b_hh, f32)
    # reorder gate rows from torch (i,f,g,o) to chunk layout (i,f,o,g)
    perm = np.concatenate([
        np.arange(0, HID), np.arange(HID, 2 * HID),
        np.arange(3 * HID, 4 * HID), np.arange(2 * HID, 3 * HID),
    ])
    Wc_r = Wc[perm]                      # [2048, 512]
    bc_r = bc[perm]                      # [2048]
    wcT = np.ascontiguousarray(Wc_r.T)   # [512 hid, 2048 gates]
    # h is carried as 16*h, so 16-bit weights are x256 (psum = raw*4096)
    wc16 = np.ascontiguousarray(
        (wcT * SW).reshape(4, 128, 4 * HID).transpose(1, 0, 2)).astype(bf16)
    # fp8 moving operands: kgroup pairs interleaved (last dim) for 2B/cyc fetch
    wc8 = np.ascontiguousarray(
        (wcT * SW).reshape(2, 2, 128, 4 * HID).transpose(2, 0, 3, 1)).astype(f8)
    woT = np.ascontiguousarray(np.asarray(W_out, f32).T)  # [512, 256]
    wo16 = np.ascontiguousarray(
        (woT * SW).reshape(4, 128, OUT_DIM).transpose(1, 0, 2)).astype(bf16)
    wo8 = np.ascontiguousarray(
        (woT * SW).reshape(2, 2, 128, OUT_DIM).transpose(2, 0, 3, 1)).astype(f8)
    winT = np.ascontiguousarray(
        (np.asarray(W_in, f32) * SH).T
        .reshape(KI, 128, HID).transpose(1, 0, 2)).astype(bf16)
    binp = np.ascontiguousarray(np.asarray(b_in, f32).reshape(4, 128).T * SH)
    biasb = np.ascontiguousarray(
        np.broadcast_to(bc_r * 4096.0, (128, 4 * HID))).astype(bf16)
    boutb = np.ascontiguousarray(np.broadcast_to(
        np.asarray(b_out, f32) * 4096.0, (128, 2, OUT_DIM))).astype(bf16)
    onesw = np.full((128, 128), 1.0 / 128.0, dtype=bf16)
    ident = np.eye(128, dtype=bf16)

    def resid8(v):
        # v*4096 == 32*(r0+r1): residual-coded fp8 pair (lhsT = 0.25*ones,
        # 128 k-rows each -> factor 128*0.25 = 32)
        tgt = np.asarray(v, f32) * 128.0
        r0 = tgt.astype(f8)
        r1 = (tgt - r0.astype(f32)).astype(f8)
        return r0, r1

    ones8 = np.full((128, 2, 128), 0.25, dtype=f8)
    r0, r1 = resid8(bc_r)
    biasb8 = np.ascontiguousarray(np.broadcast_to(
        np.stack([r0, r1], axis=-1), (128, 4 * HID, 2)))
    ro0, ro1 = resid8(np.tile(np.asarray(b_out, f32), 2))
    boutb8 = np.ascontiguousarray(np.broadcast_to(
        np.stack([ro0, ro1], axis=-1), (128, 2 * OUT_DIM, 2)))

    shared = {
        "winT": winT, "binp": binp, "wc16": wc16, "wc8": wc8,
        "wo16": wo16, "wo8": wo8, "biasb": biasb, "boutb": boutb,
        "onesw": onesw, "ones8": ones8, "biasb8": biasb8, "boutb8": boutb8,
        "ident": ident,
    }
    in_maps = []
    x = np.asarray(x, f32)
    for cid in range(NCORES):
        xs = x[cid * BSH:(cid + 1) * BSH]          # [256, 1024]
        xTc = np.ascontiguousarray(
            xs.T.reshape(KI, 128, BSH).transpose(1, 0, 2)).astype(bf16)
        in_maps.append({"xT": xTc, **shared})
    return in_maps


def kernel(x, W_in, b_in, W_ih, b_ih, W_hh, b_hh, W_out, b_out, trace=False):
    global LAST_EXEC_NS
    nc = _get_program()
    in_maps = _pack_inputs(x, W_in, b_in, W_ih, b_ih, W_hh, b_hh, W_out, b_out)
    if trace:
        trace = _install_ntff_hook()
    res = run_bass_kernel_spmd(nc, in_maps, core_ids=list(range(NCORES)), trace=trace)
    LAST_EXEC_NS = res.exec_time_ns
    return np.concatenate(
        [res.results[c]["out"].astype(np.float32) for c in range(NCORES)], axis=0)
